# revision 47
# baseline (speedup 1.0000x reference)
"""Causal multi-head attention (B=2, S=2048, D=2048, H=16) on 8 trn2 cores.

Sharding: core c handles batch b=c//4 and head-group g=c%4 (4 heads each).
Megatron-style: column-sharded Wq/Wk/Wv, row-sharded Wo; the output
projection's partial products are summed on the host (the all-reduce
equivalent), bias added on host.

Per-core kernel (Bass/Tile):
  phase P : QT/KT (head-dim-major) and V (seq-major) projections, bf16.
    RoPE on QT/KT in hd-major layout: pair-swap via a constant permutation
    matmul on PE, then qt = raw*cos + swapped*(+-sin) on DVE.  The swap
    matmuls lag one m-group behind the projection matmuls so PE never
    waits on the psum->sbuf cast.
  attention, per head:
    q-major scores -> exp(+row-sum via accum_out) -> normalize -> DMA A out.
      (only the causally-valid lower-triangle blocks are computed/written;
       the rest of A stays at the runtime's zero-initialized value)
    k-major scores -> exp -> expA^T tiles feed O^T = V^T @ expA^T on PE,
      with the AV matmuls lagging one key-block behind the score matmuls.
      O^T columns are normalized by a reciprocal row built per head from
      deferred PE transposes of ACT-broadcast reciprocal columns.
  O-proj : O_partial = (O_heads^T)^T @ Wo_shard^T, q-major, DMA out fp32.
"""

import numpy as np
import ml_dtypes
from contextlib import ExitStack

import concourse.bass as bass
import concourse.bacc as bacc
import concourse.tile as tile
import concourse.mybir as mybir
from concourse.bass_utils import run_bass_kernel_spmd

P = 128
S = 2048
D = 2048
NH = 4            # heads per core
HG = NH * P       # 512: per-core projection width
KB = 16           # key blocks (S / P)
QB = 16           # query blocks
CH = 512          # matmul moving-operand chunk
NCHUNK = S // CH  # 4
NCORES = 8
SCALE = 1.0 / float(np.sqrt(128.0))
NEG = -1.0e30
CDT = mybir.dt.bfloat16
NPCDT = ml_dtypes.bfloat16
F32 = mybir.dt.float32

_PROGRAM_CACHE = {}


def _build_consts():
    """[128, 1152] fp32: cols 0:896 k-major slide mask, 896:1024 q-major
    triangular mask, 1024:1152 identity."""
    r = np.arange(P)[:, None]
    cc = np.arange(896)[None, :]
    km = np.where(
        cc < 384,
        NEG,
        np.where(cc < 512, np.where((cc - 384) >= r, 0.0, NEG), 0.0),
    ).astype(np.float32)
    c = np.arange(P)[None, :]
    tri = np.where(c <= r, 0.0, NEG).astype(np.float32)
    ident = np.eye(P, dtype=np.float32)
    return np.concatenate([km, tri, ident], axis=1)


def _build_rope():
    """[128, 4352] bf16: cols 0:2048 cos(s*theta_{d//2}), 2048:4096 signed
    sin (-sin on even rows, +sin on odd), 4096:4224 pair-swap permutation,
    4224:4352 k-major 0/1 causal mask for the diagonal block."""
    d = np.arange(P)[:, None]
    s = np.arange(S)[None, :]
    theta = 10000.0 ** (-2.0 * (d // 2) / 128.0)
    ang = s * theta
    cos = np.cos(ang)
    sin = np.sin(ang) * np.where(d % 2 == 0, -1.0, 1.0)
    perm = np.zeros((P, P))
    perm[np.arange(P), np.arange(P) ^ 1] = 1.0
    t01 = (np.arange(P)[None, :] >= np.arange(P)[:, None]).astype(np.float64)
    return np.concatenate([cos, sin, perm, t01], axis=1).astype(NPCDT)


def _build_program():
    nc = bacc.Bacc("TRN2", target_bir_lowering=False, debug=False,
                   num_devices=NCORES)

    xT = nc.dram_tensor("xT", [D, S], CDT, kind="ExternalInput").ap()
    wqT = nc.dram_tensor("wqT", [D, HG], CDT, kind="ExternalInput").ap()
    wkT = nc.dram_tensor("wkT", [D, HG], CDT, kind="ExternalInput").ap()
    wvT = nc.dram_tensor("wvT", [D, HG], CDT, kind="ExternalInput").ap()
    woT = nc.dram_tensor("woT", [HG, D], CDT, kind="ExternalInput").ap()
    consts = nc.dram_tensor("consts", [P, 1152], F32, kind="ExternalInput").ap()
    ropec = nc.dram_tensor("ropec", [P, 4352], CDT, kind="ExternalInput").ap()
    A = nc.dram_tensor("A", [NH, S, S], F32, kind="ExternalOutput").ap()
    O = nc.dram_tensor("O", [S, D], F32, kind="ExternalOutput").ap()

    Exp = mybir.ActivationFunctionType.Exp
    Identity = mybir.ActivationFunctionType.Identity
    AXX = mybir.AxisListType.X

    with ExitStack() as ctx:
        tc = ctx.enter_context(tile.TileContext(nc))

        perm = ctx.enter_context(tc.tile_pool(name="perm", bufs=1))
        qt_sb = perm.tile([P, NH, S], CDT, name="qt_sb")   # Q^T per head
        kt_sb = perm.tile([P, NH, S], CDT, name="kt_sb")   # K^T per head
        v_sb = perm.tile([P, KB, HG], CDT, name="v_sb")    # V seq-major
        ot_sb = perm.tile([P, NH, S], CDT, name="ot_sb")   # O^T per head
        cst = perm.tile([P, 1152], F32, name="cst")
        km = cst[:, 0:896]
        tri = cst[:, 896:1024]
        ident = cst[:, 1024:1152]
        t01 = perm.tile([P, P], CDT, name="t01")
        nc.sync.dma_start(t01[:], ropec[:, 4224:4352])

        # PSUM: 4 accumulator banks + a 2-deep rotation of 2-bank score tiles
        psA = ctx.enter_context(tc.tile_pool(name="psA", bufs=1, space="PSUM"))
        psP = ctx.enter_context(tc.tile_pool(name="psP", bufs=2, space="PSUM"))

        def ps_tile(width=2 * CH):
            return psP.tile([P, 2 * CH], F32, name="ps")[:, :width]

        # ---------------- phase P: projections ----------------
        with tc.tile_pool(name="pin", bufs=1) as pin, \
             tc.tile_pool(name="wpool", bufs=2) as wpool, \
             tc.tile_pool(name="qrp", bufs=8) as qrp, \
             tc.tile_pool(name="tp", bufs=2) as tp:
            xt = pin.tile([P, KB, S], CDT, name="xt")
            rp = pin.tile([P, 4224], CDT, name="rp")
            xTr = xT.rearrange("(k p) q -> k p q", p=P)
            rpr = ropec[:, 0:4224]
            wq = wpool.tile([P, KB, HG], CDT, name="w")
            wqr = wqT.rearrange("(k p) h -> k p h", p=P)
            # interleave so tiles land in consumption order
            for k in range(KB):
                nc.sync.dma_start(wq[:, k, :], wqr[k])
                nc.sync.dma_start(xt[:, k, :], xTr[k])
            nc.sync.dma_start(rp[:], rpr)
            nc.sync.dma_start(cst[:], consts[:])
            rcos = rp[:, 0:S]
            rsin = rp[:, S:2 * S]
            rperm = rp[:, 2 * S:2 * S + P]

            # wv is loaded into the second weight-pool slot right away so the
            # V projection (run between QT and KT) never waits on DMA; wk
            # then loads under the V matmuls.
            wv = wpool.tile([P, KB, HG], CDT, name="w")
            wvr = wvT.rearrange("(k p) h -> k p h", p=P)
            for k in range(KB):
                nc.sync.dma_start(wv[:, k, :], wvr[k])

            def v_proj():
                for m in range(KB):
                    ps = ps_tile(CH)
                    for k in range(KB):
                        nc.tensor.matmul(
                            ps[:],
                            lhsT=xt[:, k, m * P:(m + 1) * P],
                            rhs=wv[:, k, :],
                            start=(k == 0), stop=(k == KB - 1))
                    nc.scalar.copy(v_sb[:, m, :], ps[:])

            # QT / KT: out[hd, q] = W[hd, :] @ x[q, :]^T, then RoPE.
            # The rope stage for m-group m is emitted after the projection
            # matmuls of group m+1 so PE's swap matmul never stalls.
            for wi, (wdr, osb) in enumerate(((wqT, qt_sb), (wkT, kt_sb))):
                if wi == 0:
                    wsb = wq
                else:
                    v_proj()
                    wsb = wpool.tile([P, KB, HG], CDT, name="w")
                    wr = wdr.rearrange("(k p) h -> k p h", p=P)
                    for k in range(KB):
                        nc.sync.dma_start(wsb[:, k, :], wr[k])

                pending = []

                def rope_stage(m, qraws, osb=osb):
                    for q in range(NCHUNK):
                        cols = slice(q * CH, (q + 1) * CH)
                        sw = ps_tile(CH)
                        nc.tensor.matmul(sw[:], lhsT=rperm, rhs=qraws[q][:],
                                         start=True, stop=True)
                        t1 = tp.tile([P, CH], F32, name="t1")
                        nc.vector.tensor_mul(t1[:], qraws[q][:], rcos[:, cols])
                        t2 = tp.tile([P, CH], F32, name="t2")
                        nc.vector.tensor_mul(t2[:], sw[:], rsin[:, cols])
                        nc.vector.tensor_add(osb[:, m, cols], t1[:], t2[:])

                for m in range(NH):
                    accs = [psA.tile([P, CH], F32, name=f"acc{q}")
                            for q in range(NCHUNK)]
                    for k in range(KB):
                        for q in range(NCHUNK):
                            nc.tensor.matmul(
                                accs[q][:],
                                lhsT=wsb[:, k, m * P:(m + 1) * P],
                                rhs=xt[:, k, q * CH:(q + 1) * CH],
                                start=(k == 0), stop=(k == KB - 1))
                    qraws = []
                    for q in range(NCHUNK):
                        qraw = qrp.tile([P, CH], CDT, name="qraw")
                        if q % 2 == 0:
                            nc.scalar.copy(qraw[:], accs[q][:])
                        else:
                            nc.vector.tensor_copy(out=qraw[:], in_=accs[q][:])
                        qraws.append(qraw)
                    pending.append((m, qraws))
                    if len(pending) > 1:
                        rope_stage(*pending.pop(0))
                rope_stage(*pending.pop(0))



        # ---------------- attention ----------------
        wop = ctx.enter_context(tc.tile_pool(name="wop", bufs=1))
        wo_sb = wop.tile([P, NH, S], CDT, name="wo_sb")
        wor = woT.rearrange("(h p) n -> h p n", p=P)
        for h in range(NH):
            nc.sync.dma_start(wo_sb[:, h, :], wor[h])

        Ep = ctx.enter_context(tc.tile_pool(name="Ep", bufs=3))
        Ap_ = ctx.enter_context(tc.tile_pool(name="Ap", bufs=2))
        eTp = ctx.enter_context(tc.tile_pool(name="eTp", bufs=3))
        scp = ctx.enter_context(tc.tile_pool(name="scp", bufs=2))
        sm = ctx.enter_context(tc.tile_pool(name="sm", bufs=4))

        for h in range(NH):
            scale_sb = scp.tile([P, S], F32, name="scale_sb")
            rcpall = sm.tile([P, QB], F32, name="rcpall")

            # ---- q-major: softmax + A output ----
            for m in range(QB):
                W = P * (m + 1)
                E = Ep.tile([P, S], F32, name="E")
                sums = sm.tile([P, 2], F32, name="sums")
                pos = 0
                kcidx = 0
                while pos < W:
                    wtile = min(2 * CH, W - pos)
                    ps = ps_tile(wtile)
                    off = 0
                    while off < wtile:
                        wmm = min(CH, wtile - off)
                        nc.tensor.matmul(
                            ps[:, off:off + wmm],
                            lhsT=qt_sb[:, h, m * P:(m + 1) * P],
                            rhs=kt_sb[:, h, pos + off:pos + off + wmm],
                            start=True, stop=True)
                        off += wmm
                    if pos + wtile == W:
                        s0 = wtile - P
                        nc.vector.tensor_add(ps[:, s0:s0 + P],
                                             ps[:, s0:s0 + P], tri)
                    nc.scalar.activation(E[:, pos:pos + wtile],
                                         ps[:], Exp, scale=SCALE,
                                         accum_out=sums[:, kcidx:kcidx + 1])
                    pos += wtile
                    kcidx += 1
                ssum = sm.tile([P, 1], F32, name="ssum")
                nc.vector.reduce_sum(ssum[:], sums[:, :kcidx], axis=AXX)
                nc.vector.reciprocal(rcpall[:, m:m + 1], ssum[:])
                Ao = Ap_.tile([P, S], F32, name="Ao")
                nc.vector.tensor_scalar_mul(Ao[:, :W], E[:, :W],
                                            rcpall[:, m:m + 1])
                nc.sync.dma_start(A[h, m * P:(m + 1) * P, 0:W], Ao[:, :W])

            # deferred: replicate each m's reciprocal column into a row of
            # scale_sb via ACT broadcast + PE transpose (emitted spread
            # through the k-major loop below so PE never blocks on them)
            def scale_row(m, scale_sb=scale_sb, rcpall=rcpall):
                bc = sm.tile([P, P], F32, name="bc")
                nc.scalar.activation(bc[:], ident, Identity,
                                     bias=rcpall[:, m:m + 1], scale=0.0)
                pst = ps_tile(P)
                nc.tensor.transpose(pst[:], bc[:], ident)
                nc.vector.tensor_copy(out=scale_sb[:, m * P:(m + 1) * P],
                                      in_=pst[:])

            # ---- k-major: expA^T and O^T = V^T @ expA^T ----
            accs = [psA.tile([P, CH], F32, name=f"acc{q}")
                    for q in range(NCHUNK)]
            eTs = {}

            def av_stage(j, h=h, accs=accs, eTs=eTs):
                eT = eTs.pop(j)
                for qc in range(j // 4, NCHUNK):
                    o = P * (j % 4) if qc == j // 4 else 0
                    nc.tensor.matmul(
                        accs[qc][:, o:],
                        lhsT=v_sb[:, j, h * P:(h + 1) * P],
                        rhs=eT[:, qc * CH + o:(qc + 1) * CH],
                        start=(j == 0), stop=(j == 4 * qc + 3))
                    if j == 4 * qc + 3:
                        nc.vector.tensor_mul(
                            ot_sb[:, h, qc * CH:(qc + 1) * CH],
                            accs[qc][:],
                            scale_sb[:, qc * CH:(qc + 1) * CH])

            for j in range(KB):
                eT = eTp.tile([P, S], CDT, name="eT")
                eTs[j] = eT
                qc0 = j // 4
                s0 = P * (j % 4)
                if s0:
                    # columns left of the diagonal are exactly zero; skip
                    # computing/exp-ing them and zero the eT prefix instead
                    nc.gpsimd.memset(eT[:, qc0 * CH:qc0 * CH + s0], 0.0)
                for qp in range(qc0, NCHUNK, 2):
                    nch = min(2, NCHUNK - qp)
                    ps = ps_tile(nch * CH)
                    off0 = s0 if qp == qc0 else 0
                    for t in range(nch):
                        o = off0 if t == 0 else t * CH
                        nc.tensor.matmul(
                            ps[:, o:(t + 1) * CH],
                            lhsT=kt_sb[:, h, j * P:(j + 1) * P],
                            rhs=qt_sb[:, h, qp * CH + o:(qp + t + 1) * CH],
                            start=True, stop=True)
                    nc.scalar.activation(
                        eT[:, qp * CH + off0:(qp + nch) * CH],
                        ps[:, off0:], Exp, scale=SCALE)
                # causal mask on the diagonal block, applied multiplicatively
                # in SBUF after exp (keeps DVE off the PSUM critical path)
                dg = qc0 * CH + s0
                nc.vector.tensor_mul(eT[:, dg:dg + P], eT[:, dg:dg + P],
                                     t01[:])
                scale_row(j)
                if j > 0:
                    av_stage(j - 1)
            av_stage(KB - 1)

        # ---------------- output projection ----------------
        op = ctx.enter_context(tc.tile_pool(name="op", bufs=6))
        for m in range(QB):
            accs = [psA.tile([P, CH], F32, name=f"acc{n}")
                    for n in range(NCHUNK)]
            for kh in range(NH):
                for n in range(NCHUNK):
                    nc.tensor.matmul(
                        accs[n][:],
                        lhsT=ot_sb[:, kh, m * P:(m + 1) * P],
                        rhs=wo_sb[:, kh, n * CH:(n + 1) * CH],
                        start=(kh == 0), stop=(kh == NH - 1))
            for n in range(NCHUNK):
                o_t = op.tile([P, CH], F32, name="o_t")
                if n % 2 == 0:
                    nc.scalar.copy(o_t[:], accs[n][:])
                else:
                    nc.vector.tensor_copy(out=o_t[:], in_=accs[n][:])
                nc.sync.dma_start(O[m * P:(m + 1) * P, n * CH:(n + 1) * CH],
                                  o_t[:])

    nc.compile()
    return nc


def _get_program():
    if "nc" not in _PROGRAM_CACHE:
        _PROGRAM_CACHE["nc"] = _build_program()
    return _PROGRAM_CACHE["nc"]


def _run(inputs, trace=False, **spmd_kwargs):
    x = np.asarray(inputs["x"], dtype=np.float32)
    Wq = np.asarray(inputs["Wq"], dtype=np.float32)
    Wk = np.asarray(inputs["Wk"], dtype=np.float32)
    Wv = np.asarray(inputs["Wv"], dtype=np.float32)
    Wo = np.asarray(inputs["Wo"], dtype=np.float32)
    bo = np.asarray(inputs["bo"], dtype=np.float32)

    consts = _build_consts()
    ropec = _build_rope()
    nc = _get_program()

    xTb = [np.ascontiguousarray(x[b].T).astype(NPCDT) for b in range(2)]
    in_maps = []
    for c in range(NCORES):
        b, g = c // 4, c % 4
        rows = slice(g * HG, (g + 1) * HG)
        in_maps.append({
            "xT": xTb[b],
            "wqT": np.ascontiguousarray(Wq[rows].T).astype(NPCDT),
            "wkT": np.ascontiguousarray(Wk[rows].T).astype(NPCDT),
            "wvT": np.ascontiguousarray(Wv[rows].T).astype(NPCDT),
            "woT": np.ascontiguousarray(Wo[:, rows].T).astype(NPCDT),
            "consts": consts,
            "ropec": ropec,
        })

    res = run_bass_kernel_spmd(nc, in_maps, list(range(NCORES)),
                               trace=trace, **spmd_kwargs)

    A = np.empty((2, 16, S, S), dtype=np.float32)
    attn = np.zeros((2, S, D), dtype=np.float32)
    for c, out in enumerate(res.results):
        b, g = c // 4, c % 4
        A[b, g * NH:(g + 1) * NH] = out["A"]
        attn[b] += out["O"]
    attn += bo[None, None, :]
    return (attn, A), res


def kernel(**inputs):
    return _run(inputs)[0]


# revision 52
# speedup vs baseline: 1.1736x; 1.1736x over previous
"""Causal multi-head attention (B=2, S=2048, D=2048, H=16) on 8 trn2 cores.

Sharding: core c handles batch b=c//4 and head-group g=c%4 (4 heads each).
Megatron-style: column-sharded Wq/Wk/Wv, row-sharded Wo; the output
projection's partial products are summed on the host (the all-reduce
equivalent), bias added on host.

Per-core kernel (Bass/Tile):
  phase P : QT/KT (head-dim-major) and V (seq-major) projections, bf16.
    RoPE on QT/KT in hd-major layout: pair-swap via a constant permutation
    matmul on PE, then qt = raw*cos + swapped*(+-sin) on DVE.  The swap
    matmuls lag one m-group behind the projection matmuls so PE never
    waits on the psum->sbuf cast.
  attention, per head:
    q-major scores -> exp(+row-sum via accum_out) -> normalize -> DMA A out.
      (only the causally-valid lower-triangle blocks are computed/written;
       the rest of A stays at the runtime's zero-initialized value)
    k-major scores -> exp -> expA^T tiles feed O^T = V^T @ expA^T on PE,
      with the AV matmuls lagging one key-block behind the score matmuls.
      O^T columns are normalized by a reciprocal row built per head from
      deferred PE transposes of ACT-broadcast reciprocal columns.
  O-proj : O_partial = (O_heads^T)^T @ Wo_shard^T, q-major, DMA out fp32.
"""

import numpy as np
import ml_dtypes
from contextlib import ExitStack

import concourse.bass as bass
import concourse.bacc as bacc
import concourse.tile as tile
import concourse.mybir as mybir
from concourse.bass_utils import run_bass_kernel_spmd

P = 128
S = 2048
D = 2048
NH = 4            # heads per core
HG = NH * P       # 512: per-core projection width
KB = 16           # key blocks (S / P)
QB = 16           # query blocks
CH = 512          # matmul moving-operand chunk
NCHUNK = S // CH  # 4
NCORES = 8
SCALE = 1.0 / float(np.sqrt(128.0))
NEG = -1.0e30
CDT = mybir.dt.bfloat16
NPCDT = ml_dtypes.bfloat16
F32 = mybir.dt.float32

_PROGRAM_CACHE = {}


def _build_consts():
    """[128, 1152] fp32: cols 0:896 k-major slide mask, 896:1024 q-major
    triangular mask, 1024:1152 identity."""
    r = np.arange(P)[:, None]
    cc = np.arange(896)[None, :]
    km = np.where(
        cc < 384,
        NEG,
        np.where(cc < 512, np.where((cc - 384) >= r, 0.0, NEG), 0.0),
    ).astype(np.float32)
    c = np.arange(P)[None, :]
    tri = np.where(c <= r, 0.0, NEG).astype(np.float32)
    ident = np.eye(P, dtype=np.float32)
    return np.concatenate([km, tri, ident], axis=1)


def _build_rope():
    """[128, 4224] bf16: cols 0:2048 cos(s*theta_{d//2}), 2048:4096 signed
    sin (-sin on even rows, +sin on odd), 4096:4224 pair-swap permutation."""
    d = np.arange(P)[:, None]
    s = np.arange(S)[None, :]
    theta = 10000.0 ** (-2.0 * (d // 2) / 128.0)
    ang = s * theta
    cos = np.cos(ang)
    sin = np.sin(ang) * np.where(d % 2 == 0, -1.0, 1.0)
    perm = np.zeros((P, P))
    perm[np.arange(P), np.arange(P) ^ 1] = 1.0
    return np.concatenate([cos, sin, perm], axis=1).astype(NPCDT)


def _build_program():
    nc = bacc.Bacc("TRN2", target_bir_lowering=False, debug=False,
                   num_devices=NCORES)

    xT = nc.dram_tensor("xT", [D, S], CDT, kind="ExternalInput").ap()
    wqT = nc.dram_tensor("wqT", [D, HG], CDT, kind="ExternalInput").ap()
    wkT = nc.dram_tensor("wkT", [D, HG], CDT, kind="ExternalInput").ap()
    wvT = nc.dram_tensor("wvT", [D, HG], CDT, kind="ExternalInput").ap()
    woT = nc.dram_tensor("woT", [HG, D], CDT, kind="ExternalInput").ap()
    consts = nc.dram_tensor("consts", [P, 1152], F32, kind="ExternalInput").ap()
    ropec = nc.dram_tensor("ropec", [P, 4224], CDT, kind="ExternalInput").ap()
    A = nc.dram_tensor("A", [NH, S, S], F32, kind="ExternalOutput").ap()
    O = nc.dram_tensor("O", [S, D], F32, kind="ExternalOutput").ap()

    Exp = mybir.ActivationFunctionType.Exp
    Identity = mybir.ActivationFunctionType.Identity
    AXX = mybir.AxisListType.X

    with ExitStack() as ctx:
        tc = ctx.enter_context(tile.TileContext(nc))

        perm = ctx.enter_context(tc.tile_pool(name="perm", bufs=1))
        qt_sb = perm.tile([P, NH, S], CDT, name="qt_sb")   # Q^T per head
        kt_sb = perm.tile([P, NH, S], CDT, name="kt_sb")   # K^T per head
        v_sb = perm.tile([P, KB, HG], CDT, name="v_sb")    # V seq-major
        ot_sb = perm.tile([P, NH, S], CDT, name="ot_sb")   # O^T per head
        cst = perm.tile([P, 1152], F32, name="cst")
        km = cst[:, 0:896]
        tri = cst[:, 896:1024]
        ident = cst[:, 1024:1152]

        # PSUM: 4 accumulator banks + a 2-deep rotation of 2-bank score tiles
        psA = ctx.enter_context(tc.tile_pool(name="psA", bufs=1, space="PSUM"))
        psP = ctx.enter_context(tc.tile_pool(name="psP", bufs=2, space="PSUM"))

        def ps_tile(width=2 * CH):
            return psP.tile([P, 2 * CH], F32, name="ps")[:, :width]

        def ps_small(width=CH):
            # narrow score tiles borrow the acc0 bank: the AV accumulator
            # for q-chunk 0 is released at j=3, and is entirely free during
            # the q-major section, so these generations interleave cleanly
            # and add a third slot to the score-tile rotation
            return psA.tile([P, CH], F32, name="acc0")[:, :width]

        # ---------------- phase P: projections ----------------
        with tc.tile_pool(name="pin", bufs=1) as pin, \
             tc.tile_pool(name="wpool", bufs=2) as wpool, \
             tc.tile_pool(name="qrp", bufs=8) as qrp, \
             tc.tile_pool(name="tp", bufs=2) as tp:
            xt = pin.tile([P, KB, S], CDT, name="xt")
            rp = pin.tile([P, 4224], CDT, name="rp")
            xTr = xT.rearrange("(k p) q -> k p q", p=P)
            wq = wpool.tile([P, KB, HG], CDT, name="w")
            wqr = wqT.rearrange("(k p) h -> k p h", p=P)
            # interleave so tiles land in consumption order
            for k in range(KB):
                nc.sync.dma_start(wq[:, k, :], wqr[k])
                nc.sync.dma_start(xt[:, k, :], xTr[k])
            nc.sync.dma_start(rp[:], ropec[:])
            nc.sync.dma_start(cst[:], consts[:])
            rcos = rp[:, 0:S]
            rsin = rp[:, S:2 * S]
            rperm = rp[:, 2 * S:2 * S + P]

            # wv is loaded into the second weight-pool slot right away so the
            # V projection (run between QT and KT) never waits on DMA; wk
            # then loads under the V matmuls.
            wv = wpool.tile([P, KB, HG], CDT, name="w")
            wvr = wvT.rearrange("(k p) h -> k p h", p=P)
            for k in range(KB):
                nc.sync.dma_start(wv[:, k, :], wvr[k])

            def v_proj():
                for m in range(KB):
                    ps = ps_tile(CH)
                    for k in range(KB):
                        nc.tensor.matmul(
                            ps[:],
                            lhsT=xt[:, k, m * P:(m + 1) * P],
                            rhs=wv[:, k, :],
                            start=(k == 0), stop=(k == KB - 1))
                    nc.scalar.copy(v_sb[:, m, :], ps[:])

            # QT / KT: out[hd, q] = W[hd, :] @ x[q, :]^T, then RoPE.
            # The rope stage for m-group m is emitted after the projection
            # matmuls of group m+1 so PE's swap matmul never stalls.
            for wi, (wdr, osb) in enumerate(((wqT, qt_sb), (wkT, kt_sb))):
                if wi == 0:
                    wsb = wq
                else:
                    v_proj()
                    wsb = wpool.tile([P, KB, HG], CDT, name="w")
                    wr = wdr.rearrange("(k p) h -> k p h", p=P)
                    for k in range(KB):
                        nc.sync.dma_start(wsb[:, k, :], wr[k])

                pending = []

                def rope_stage(m, qraws, osb=osb):
                    for q in range(NCHUNK):
                        cols = slice(q * CH, (q + 1) * CH)
                        sw = ps_tile(CH)
                        nc.tensor.matmul(sw[:], lhsT=rperm, rhs=qraws[q][:],
                                         start=True, stop=True)
                        t1 = tp.tile([P, CH], F32, name="t1")
                        nc.vector.tensor_mul(t1[:], qraws[q][:], rcos[:, cols])
                        t2 = tp.tile([P, CH], F32, name="t2")
                        nc.vector.tensor_mul(t2[:], sw[:], rsin[:, cols])
                        nc.vector.tensor_add(osb[:, m, cols], t1[:], t2[:])

                for m in range(NH):
                    accs = [psA.tile([P, CH], F32, name=f"acc{q}")
                            for q in range(NCHUNK)]
                    for k in range(KB):
                        for q in range(NCHUNK):
                            nc.tensor.matmul(
                                accs[q][:],
                                lhsT=wsb[:, k, m * P:(m + 1) * P],
                                rhs=xt[:, k, q * CH:(q + 1) * CH],
                                start=(k == 0), stop=(k == KB - 1))
                    qraws = []
                    for q in range(NCHUNK):
                        qraw = qrp.tile([P, CH], CDT, name="qraw")
                        if q % 2 == 0:
                            nc.scalar.copy(qraw[:], accs[q][:])
                        else:
                            nc.vector.tensor_copy(out=qraw[:], in_=accs[q][:])
                        qraws.append(qraw)
                    pending.append((m, qraws))
                    if len(pending) > 1:
                        rope_stage(*pending.pop(0))
                rope_stage(*pending.pop(0))



        # ---------------- attention ----------------
        wop = ctx.enter_context(tc.tile_pool(name="wop", bufs=1))
        wo_sb = wop.tile([P, NH, S], CDT, name="wo_sb")
        wor = woT.rearrange("(h p) n -> h p n", p=P)
        for h in range(NH):
            nc.sync.dma_start(wo_sb[:, h, :], wor[h])

        Ep = ctx.enter_context(tc.tile_pool(name="Ep", bufs=3))
        Ap_ = ctx.enter_context(tc.tile_pool(name="Ap", bufs=2))
        eTp = ctx.enter_context(tc.tile_pool(name="eTp", bufs=4))
        scp = ctx.enter_context(tc.tile_pool(name="scp", bufs=2))
        sm = ctx.enter_context(tc.tile_pool(name="sm", bufs=4))

        for h in range(NH):
            scale_sb = scp.tile([P, S], F32, name="scale_sb")
            rcpall = sm.tile([P, QB], F32, name="rcpall")

            # ---- q-major: softmax + A output ----
            for m in range(QB):
                W = P * (m + 1)
                E = Ep.tile([P, S], F32, name="E")
                sums = sm.tile([P, 2], F32, name="sums")
                pos = 0
                kcidx = 0
                while pos < W:
                    wtile = min(2 * CH, W - pos)
                    ps = ps_small(wtile) if wtile <= CH else ps_tile(wtile)
                    off = 0
                    while off < wtile:
                        wmm = min(CH, wtile - off)
                        nc.tensor.matmul(
                            ps[:, off:off + wmm],
                            lhsT=qt_sb[:, h, m * P:(m + 1) * P],
                            rhs=kt_sb[:, h, pos + off:pos + off + wmm],
                            start=True, stop=True)
                        off += wmm
                    if pos + wtile == W:
                        s0 = wtile - P
                        nc.vector.tensor_add(ps[:, s0:s0 + P],
                                             ps[:, s0:s0 + P], tri)
                    nc.scalar.activation(E[:, pos:pos + wtile],
                                         ps[:], Exp, scale=SCALE,
                                         accum_out=sums[:, kcidx:kcidx + 1])
                    pos += wtile
                    kcidx += 1
                ssum = sm.tile([P, 1], F32, name="ssum")
                nc.vector.reduce_sum(ssum[:], sums[:, :kcidx], axis=AXX)
                nc.vector.reciprocal(rcpall[:, m:m + 1], ssum[:])
                Ao = Ap_.tile([P, S], F32, name="Ao")
                nc.vector.tensor_scalar_mul(Ao[:, :W], E[:, :W],
                                            rcpall[:, m:m + 1])
                nc.sync.dma_start(A[h, m * P:(m + 1) * P, 0:W], Ao[:, :W])

            # deferred: replicate each m's reciprocal column into a row of
            # scale_sb via ACT broadcast + PE transpose (emitted spread
            # through the k-major loop below so PE never blocks on them)
            def scale_row(m, scale_sb=scale_sb, rcpall=rcpall):
                bc = sm.tile([P, P], F32, name="bc")
                nc.scalar.activation(bc[:], ident, Identity,
                                     bias=rcpall[:, m:m + 1], scale=0.0)
                pst = ps_tile(P)
                nc.tensor.transpose(pst[:], bc[:], ident)
                nc.vector.tensor_copy(out=scale_sb[:, m * P:(m + 1) * P],
                                      in_=pst[:])

            # ---- k-major: expA^T and O^T = V^T @ expA^T ----
            accs = [psA.tile([P, CH], F32, name=f"acc{q}")
                    for q in range(NCHUNK)]
            eTs = {}

            def av_stage(j, h=h, accs=accs, eTs=eTs):
                eT = eTs.pop(j)
                for qc in range(j // 4, NCHUNK):
                    o = P * (j % 4) if qc == j // 4 else 0
                    nc.tensor.matmul(
                        accs[qc][:, o:],
                        lhsT=v_sb[:, j, h * P:(h + 1) * P],
                        rhs=eT[:, qc * CH + o:(qc + 1) * CH],
                        start=(j == 0), stop=(j == 4 * qc + 3))
                    if j == 4 * qc + 3:
                        nc.vector.tensor_mul(
                            ot_sb[:, h, qc * CH:(qc + 1) * CH],
                            accs[qc][:],
                            scale_sb[:, qc * CH:(qc + 1) * CH])

            for j in range(KB):
                eT = eTp.tile([P, S], CDT, name="eT")
                eTs[j] = eT
                qc0 = j // 4
                s0 = P * (j % 4)
                if s0:
                    # columns left of the diagonal are exactly zero; skip
                    # computing/exp-ing them and zero the eT prefix instead
                    nc.gpsimd.memset(eT[:, qc0 * CH:qc0 * CH + s0], 0.0)
                for qp in range(qc0, NCHUNK, 2):
                    nch = min(2, NCHUNK - qp)
                    ps = ps_small(CH) if nch == 1 else ps_tile(nch * CH)
                    off0 = s0 if qp == qc0 else 0
                    for t in range(nch):
                        o = off0 if t == 0 else t * CH
                        nc.tensor.matmul(
                            ps[:, o:(t + 1) * CH],
                            lhsT=kt_sb[:, h, j * P:(j + 1) * P],
                            rhs=qt_sb[:, h, qp * CH + o:(qp + t + 1) * CH],
                            start=True, stop=True)
                    if qp == qc0:
                        nc.vector.tensor_add(ps[:, s0:s0 + P],
                                             ps[:, s0:s0 + P],
                                             km[:, 384:384 + P])
                    nc.scalar.activation(
                        eT[:, qp * CH + off0:(qp + nch) * CH],
                        ps[:, off0:], Exp, scale=SCALE)
                scale_row(j)
                if j > 0:
                    av_stage(j - 1)
            av_stage(KB - 1)

        # ---------------- output projection ----------------
        op = ctx.enter_context(tc.tile_pool(name="op", bufs=6))
        for m in range(QB):
            accs = [psA.tile([P, CH], F32, name=f"acc{n}")
                    for n in range(NCHUNK)]
            for kh in range(NH):
                for n in range(NCHUNK):
                    nc.tensor.matmul(
                        accs[n][:],
                        lhsT=ot_sb[:, kh, m * P:(m + 1) * P],
                        rhs=wo_sb[:, kh, n * CH:(n + 1) * CH],
                        start=(kh == 0), stop=(kh == NH - 1))
            for n in range(NCHUNK):
                o_t = op.tile([P, CH], F32, name="o_t")
                if n % 2 == 0:
                    nc.scalar.copy(o_t[:], accs[n][:])
                else:
                    nc.vector.tensor_copy(out=o_t[:], in_=accs[n][:])
                nc.sync.dma_start(O[m * P:(m + 1) * P, n * CH:(n + 1) * CH],
                                  o_t[:])

    nc.compile()
    return nc


def _get_program():
    if "nc" not in _PROGRAM_CACHE:
        _PROGRAM_CACHE["nc"] = _build_program()
    return _PROGRAM_CACHE["nc"]


def _run(inputs, trace=False, **spmd_kwargs):
    x = np.asarray(inputs["x"], dtype=np.float32)
    Wq = np.asarray(inputs["Wq"], dtype=np.float32)
    Wk = np.asarray(inputs["Wk"], dtype=np.float32)
    Wv = np.asarray(inputs["Wv"], dtype=np.float32)
    Wo = np.asarray(inputs["Wo"], dtype=np.float32)
    bo = np.asarray(inputs["bo"], dtype=np.float32)

    consts = _build_consts()
    ropec = _build_rope()
    nc = _get_program()

    xTb = [np.ascontiguousarray(x[b].T).astype(NPCDT) for b in range(2)]
    in_maps = []
    for c in range(NCORES):
        b, g = c // 4, c % 4
        rows = slice(g * HG, (g + 1) * HG)
        in_maps.append({
            "xT": xTb[b],
            "wqT": np.ascontiguousarray(Wq[rows].T).astype(NPCDT),
            "wkT": np.ascontiguousarray(Wk[rows].T).astype(NPCDT),
            "wvT": np.ascontiguousarray(Wv[rows].T).astype(NPCDT),
            "woT": np.ascontiguousarray(Wo[:, rows].T).astype(NPCDT),
            "consts": consts,
            "ropec": ropec,
        })

    res = run_bass_kernel_spmd(nc, in_maps, list(range(NCORES)),
                               trace=trace, **spmd_kwargs)

    A = np.empty((2, 16, S, S), dtype=np.float32)
    attn = np.zeros((2, S, D), dtype=np.float32)
    for c, out in enumerate(res.results):
        b, g = c // 4, c % 4
        A[b, g * NH:(g + 1) * NH] = out["A"]
        attn[b] += out["O"]
    attn += bo[None, None, :]
    return (attn, A), res


def kernel(**inputs):
    return _run(inputs)[0]
